# revision 34
# baseline (speedup 1.0000x reference)
"""Trainium2 Bass kernel for nn_Attention_23424751632639.

Computation (per (b,h)):  out = tril_strict(rope(Q) @ rope(Q).T / sqrt(N)) @ V
Reformulated as chunked linear attention (exact, just reordered sums):
  out_c = QR_c @ M_c  +  strict_mask(QR_c @ QR_c^T) @ V_c
  M_{c+1} = M_c + QR_c^T @ V_c            (M is the [64,64] running state)
with QR = rope(Q) * N**-0.25 (scale folded into the cos/sin tables, so the
score scale N**-0.5 appears automatically in both the intra and inter terms).

RoPE is computed as  QR = Q*CC + swap(Q)*SS  where swap exchanges feature
pairs (2m <-> 2m+1) and the rotation sign is folded into SS.  swap runs on
GpSimd (otherwise idle), the three wide elementwise ops on DVE.

Matmul operands are bf16 (PE runs 1 cyc/row vs 4 for fp32); all accumulation
(PSUM, the M state) stays fp32.  QR^T is built by xbar DMA transposes
(2-byte dtype), one per chunk pair, written straight into the [64, T] strip
via a 3D destination AP.

Sharding: B*H = 32 (b,h) pairs -> 4 per core across 8 cores; no collectives.
"""

import math
import sys

import numpy as np

if "/opt/trn_rl_repo" not in sys.path:
    sys.path.insert(0, "/opt/trn_rl_repo")

B, H, T, N = 2, 16, 4096, 64
THETA = 2.0 ** 16
NCORES = 8
HPC = (B * H) // NCORES  # heads per core
QTR = 8  # chunks per pipeline stage (quarter-head granularity)


def _host_tables(t_len):
    """Full-width scaled RoPE tables CC, SS [t_len, N] float32.

    QR[t,n] = Q[t,n]*CC[t,n] + swap(Q)[t,n]*SS[t,n]
    where swap(Q)[2m] = Q[2m+1], swap(Q)[2m+1] = Q[2m]; the rotation minus
    sign lives in SS's even columns.
    """
    n = np.arange(N, dtype=np.float64)
    tq = np.floor(n / 2.0) * 2.0
    freqs = 1.0 / (THETA ** (tq / N)) / (2.0 * math.pi)  # [N]
    t = np.arange(t_len, dtype=np.float64)[:, None]
    ang = ((t * freqs[None, :]) % 1.0) * (2.0 * math.pi)  # [t_len, N]
    scale = float(N) ** -0.25
    cc = (np.cos(ang) * scale).astype(np.float32)
    ss = (np.sin(ang) * scale).astype(np.float32)
    ss[:, 0::2] *= -1.0
    return np.ascontiguousarray(cc), np.ascontiguousarray(ss)


def build_program(t_len=T, hpc=HPC, sim_init=False):
    import concourse.mybir as mybir
    import concourse.tile as tile
    from concourse import bacc
    from concourse.tile_rust import add_dep_helper

    f32 = mybir.dt.float32
    bf = mybir.dt.bfloat16
    ch = t_len // 128  # number of 128-row chunks per head
    qtr = min(QTR, ch)  # chunks per pipeline stage
    nq = ch // qtr  # pipeline stages per head

    # race detection can't model the xbar transpose's 3D dst AP (deps for it
    # are added manually below), so it is disabled
    nc = bacc.Bacc(None, target_bir_lowering=False, detect_race_conditions=False)
    q = nc.dram_tensor("q", [hpc, t_len, N], f32, kind="ExternalInput")
    v = nc.dram_tensor("v", [hpc, t_len, N], f32, kind="ExternalInput")
    cc = nc.dram_tensor("cc", [t_len, N], f32, kind="ExternalInput")
    ss = nc.dram_tensor("ss", [t_len, N], f32, kind="ExternalInput")
    mu = nc.dram_tensor("mu", [128, 128], f32, kind="ExternalInput")
    o = nc.dram_tensor("o", [hpc, t_len, N], f32, kind="ExternalOutput")

    with tile.TileContext(nc) as tc:
        with (
            tc.tile_pool(name="const", bufs=1) as constp,
            tc.tile_pool(name="head", bufs=2) as headp,
            tc.tile_pool(name="rope", bufs=3) as ropep,
            tc.tile_pool(name="work", bufs=3) as workp,
            tc.tile_pool(name="ps", bufs=3, space="PSUM") as psp,
            tc.tile_pool(name="psm", bufs=2, space="PSUM") as psmp,
        ):
            cc_sb = constp.tile([128, ch * N], f32)
            ss_sb = constp.tile([128, ch * N], f32)
            mu_sb = constp.tile([128, 128], f32)
            nc.sync.dma_start(
                cc_sb.rearrange("p (c n) -> p c n", c=ch),
                cc.rearrange("(c p) n -> p c n", p=128),
            )
            nc.sync.dma_start(
                ss_sb.rearrange("p (c n) -> p c n", c=ch),
                ss.rearrange("(c p) n -> p c n", p=128),
            )
            nc.sync.dma_start(mu_sb[:], mu[:])

            prev_qrt_reader = None
            for h in range(hpc):
                q_sb = headp.tile([128, ch * N], f32, tag="q")
                v_sb = headp.tile([128, ch * N], bf, tag="v")
                qr = headp.tile([128, ch * N], bf, tag="qr")
                # split-pair layout: pair cp at cols [cp*128,(cp+1)*128),
                # chunk 2cp on partitions 0:64, chunk 2cp+1 on 64:128
                qrt = headp.tile([128, ch * 64], bf, tag="qrt")

                q3 = q_sb.rearrange("p (c n) -> p c n", c=ch)
                v3 = v_sb.rearrange("p (c n) -> p c n", c=ch)
                tr_insts = {}  # chunk pair -> transpose DMA (manual RAW deps)
                qrt_init = None
                if sim_init:
                    # CoreSim's init tracker can't see the xbar transpose's
                    # 3D dst AP; pre-initialize qrt in sim builds only.
                    qrt_init = nc.vector.memset(qrt[:], 0.0)

                # load + rope + transpose, pipelined per quarter-head
                for qt in range(nq):
                    csl = slice(qt * qtr, (qt + 1) * qtr)
                    fsl = slice(qt * qtr * N, (qt + 1) * qtr * N)
                    nc.sync.dma_start(q3[:, csl], q[h].rearrange(
                        "(c p) n -> p c n", p=128)[:, csl])
                    # SWDGE casts f32 -> bf16 during the transfer
                    nc.gpsimd.dma_start(v3[:, csl], v[h].rearrange(
                        "(c p) n -> p c n", p=128)[:, csl])

                    # swap(Q): exchange feature pairs, cast to bf16 (GpSimd)
                    swp = ropep.tile([128, qtr * N], bf, tag="swp")
                    sw4 = swp.rearrange("p (c m o) -> p c m o", c=qtr, m=32, o=2)
                    q4 = q3[:, csl].rearrange("p c (m o) -> p c m o", m=32)
                    nc.gpsimd.tensor_copy(sw4[:, :, :, 0], q4[:, :, :, 1])
                    nc.gpsimd.tensor_copy(sw4[:, :, :, 1], q4[:, :, :, 0])

                    # QR = Q*CC + swap(Q)*SS   (contiguous DVE ops)
                    t1 = ropep.tile([128, qtr * N], f32, tag="t1")
                    t2 = ropep.tile([128, qtr * N], f32, tag="t2")
                    nc.vector.tensor_mul(t1[:], q_sb[:, fsl], cc_sb[:, fsl])
                    nc.vector.tensor_mul(t2[:], swp[:], ss_sb[:, fsl])
                    nc.vector.tensor_add(qr[:, fsl], t1[:], t2[:])

                    # QR^T strips via xbar transpose, one per chunk pair
                    for cp in range(qt * qtr // 2, (qt + 1) * qtr // 2):
                        tr_insts[cp] = nc.sync.dma_start(
                            qrt[:, cp * 128:(cp + 1) * 128],
                            qr[:, cp * 128:(cp + 1) * 128],
                            transpose=True,
                        )
                        if qrt_init is not None:
                            add_dep_helper(tr_insts[cp].ins, qrt_init.ins,
                                           reason="memset before xbar write")
                        if prev_qrt_reader is not None:
                            # WAW guard: qrt slot reuse across heads is not
                            # visible to the tracker either
                            add_dep_helper(tr_insts[cp].ins,
                                           prev_qrt_reader.ins,
                                           reason="qrt slot WAR across heads")

                m_ps = psmp.tile([128, 64], f32, tag="m")  # state, both halves
                mb_prev = None
                ost = None
                for c in range(ch):
                    qr_c = qr[:, c * 64:(c + 1) * 64]
                    v_c = v_sb[:, c * 64:(c + 1) * 64]
                    half = slice(64 * (c % 2), 64 * (c % 2) + 64)
                    qrt_c = qrt[half, (c // 2) * 128:(c // 2 + 1) * 128]

                    # intra: P = QR_c @ QR_c^T, then strict-upper mask as lhsT
                    # (the xbar transpose's 3D dst AP isn't dep-tracked, so
                    # RAW edges on qrt are added explicitly)
                    p_ps = psp.tile([128, 128], f32, tag="p")
                    mm1 = nc.tensor.matmul(
                        p_ps[:], qrt_c, qrt_c, start=True, stop=True
                    )
                    add_dep_helper(mm1.ins, tr_insts[c // 2].ins,
                                   reason="qrt xbar RAW")
                    if c == ch - 1:
                        prev_qrt_reader = mm1
                    p_sb = workp.tile([128, 128], bf, tag="psb")
                    nc.vector.tensor_mul(p_sb[:], p_ps[:], mu_sb[:])

                    out_ps = psp.tile([128, 64], f32, tag="out")
                    if c == 0:
                        nc.tensor.matmul(out_ps[:], p_sb[:], v_c, start=True, stop=True)
                    else:
                        # inter: out += QR_c @ M   (M = state after chunk c-1)
                        mmi = nc.tensor.matmul(
                            out_ps[:], qrt_c, mb_prev[half, :],
                            start=True, stop=False,
                        )
                        add_dep_helper(mmi.ins, tr_insts[c // 2].ins,
                                       reason="qrt xbar RAW")
                        nc.tensor.matmul(
                            out_ps[:], p_sb[:], v_c, start=False, stop=True
                        )

                    # state: M += QR_c^T @ V_c, accumulated in PSUM in
                    # both partition halves (inter-mm needs M at the same
                    # base partition as its lhsT from the split-pair qrt)
                    nc.tensor.matmul(
                        m_ps[0:64, :], qr_c, v_c,
                        start=(c == 0), stop=(c == ch - 1),
                        skip_group_check=True,
                    )
                    nc.tensor.matmul(
                        m_ps[64:128, :], qr_c, v_c,
                        start=(c == 0), stop=(c == ch - 1),
                        skip_group_check=True,
                    )
                    if c < ch - 1:
                        m_bf = workp.tile([128, 64], bf, tag="mbf")
                        nc.scalar.copy(m_bf[:], m_ps[:])
                        mb_prev = m_bf

                    # batch output: stage 4 chunks, then one DMA
                    k = c % 4
                    if k == 0:
                        ost = workp.tile([128, 256], f32, tag="ost")
                    nc.scalar.copy(ost[:, k * 64:(k + 1) * 64], out_ps[:])
                    if k == 3:
                        nc.sync.dma_start(
                            o[h].rearrange("(g p) n -> p g n", p=128)[
                                :, c // 4 * 4:c // 4 * 4 + 4],
                            ost.rearrange("p (g n) -> p g n", g=4),
                        )

    nc.compile()
    return nc


_CACHE = {}


def _get_program():
    if "nc" not in _CACHE:
        _CACHE["nc"] = build_program()
    return _CACHE["nc"]


def _strict_upper_mask():
    # lhsT for the diag block: keep P[j, i] where j < i
    return np.triu(np.ones((128, 128), dtype=np.float32), k=1)


def kernel(Q, V):
    from concourse.bass_utils import run_bass_kernel_spmd

    Q = np.ascontiguousarray(np.asarray(Q), dtype=np.float32)
    V = np.ascontiguousarray(np.asarray(V), dtype=np.float32)
    qf = Q.reshape(NCORES, HPC, T, N)
    vf = V.reshape(NCORES, HPC, T, N)
    cc, ss = _host_tables(T)
    mu = _strict_upper_mask()

    nc = _get_program()
    in_maps = [
        {"q": qf[i], "v": vf[i], "cc": cc, "ss": ss, "mu": mu}
        for i in range(NCORES)
    ]
    res = run_bass_kernel_spmd(nc, in_maps, core_ids=list(range(NCORES)))
    out = np.stack([r["o"] for r in res.results], axis=0)
    return out.reshape(B, H, T, N)


# revision 36
# speedup vs baseline: 1.3852x; 1.3852x over previous
"""Trainium2 Bass kernel for nn_Attention_23424751632639.

Computation (per (b,h)):  out = tril_strict(rope(Q) @ rope(Q).T / sqrt(N)) @ V
Reformulated as chunked linear attention (exact, just reordered sums):
  out_c = QR_c @ M_c  +  strict_mask(QR_c @ QR_c^T) @ V_c
  M_{c+1} = M_c + QR_c^T @ V_c            (M is the [64,64] running state)
with QR = rope(Q) * N**-0.25 (scale folded into the cos/sin tables, so the
score scale N**-0.5 appears automatically in both the intra and inter terms).

RoPE is computed as  QR = Q*CC + swap(Q)*SS  where swap exchanges feature
pairs (2m <-> 2m+1) and the rotation sign is folded into SS.  swap runs on
GpSimd (otherwise idle), the three wide elementwise ops on DVE.

Matmul operands are bf16 (PE 1 cyc/row vs 4 for fp32); all accumulation
(PSUM, the M state) stays fp32.  QR^T strips come from PE transposes.
Chunks are processed in pairs sharing PSUM tiles so each DVE/ACT fixup op
runs once per pair at double width.

Sharding: B*H = 32 (b,h) pairs -> 4 per core across 8 cores; no collectives.
"""

import math
import sys

import numpy as np

if "/opt/trn_rl_repo" not in sys.path:
    sys.path.insert(0, "/opt/trn_rl_repo")

B, H, T, N = 2, 16, 4096, 64
THETA = 2.0 ** 16
NCORES = 8
HPC = (B * H) // NCORES  # heads per core
QTR = 8  # chunks per pipeline stage (quarter-head granularity)


def _host_tables(t_len):
    """Full-width scaled RoPE tables CC, SS [t_len, N] float32.

    QR[t,n] = Q[t,n]*CC[t,n] + swap(Q)[t,n]*SS[t,n]
    where swap(Q)[2m] = Q[2m+1], swap(Q)[2m+1] = Q[2m]; the rotation minus
    sign lives in SS's even columns.
    """
    n = np.arange(N, dtype=np.float64)
    tq = np.floor(n / 2.0) * 2.0
    freqs = 1.0 / (THETA ** (tq / N)) / (2.0 * math.pi)  # [N]
    t = np.arange(t_len, dtype=np.float64)[:, None]
    ang = ((t * freqs[None, :]) % 1.0) * (2.0 * math.pi)  # [t_len, N]
    scale = float(N) ** -0.25
    cc = (np.cos(ang) * scale).astype(np.float32)
    ss = (np.sin(ang) * scale).astype(np.float32)
    ss[:, 0::2] *= -1.0
    return np.ascontiguousarray(cc), np.ascontiguousarray(ss)


def build_program(t_len=T, hpc=HPC):
    import concourse.mybir as mybir
    import concourse.tile as tile
    from concourse import bacc

    f32 = mybir.dt.float32
    bf = mybir.dt.bfloat16
    ch = t_len // 128  # number of 128-row chunks per head
    qtr = min(QTR, ch)  # chunks per pipeline stage
    nq = ch // qtr  # pipeline stages per head

    nc = bacc.Bacc(None, target_bir_lowering=False)
    q = nc.dram_tensor("q", [hpc, t_len, N], f32, kind="ExternalInput")
    v = nc.dram_tensor("v", [hpc, t_len, N], f32, kind="ExternalInput")
    cc = nc.dram_tensor("cc", [t_len, N], f32, kind="ExternalInput")
    ss = nc.dram_tensor("ss", [t_len, N], f32, kind="ExternalInput")
    mu = nc.dram_tensor("mu", [128, 256], f32, kind="ExternalInput")
    ident = nc.dram_tensor("ident", [128, 128], bf, kind="ExternalInput")
    o = nc.dram_tensor("o", [hpc, t_len, N], f32, kind="ExternalOutput")

    with tile.TileContext(nc) as tc:
        with (
            tc.tile_pool(name="const", bufs=1) as constp,
            tc.tile_pool(name="head", bufs=2) as headp,
            tc.tile_pool(name="rope", bufs=3) as ropep,
            tc.tile_pool(name="work", bufs=3) as workp,
            tc.tile_pool(name="ps", bufs=2, space="PSUM") as psp,
            tc.tile_pool(name="psm", bufs=2, space="PSUM") as psmp,
        ):
            cc_sb = constp.tile([128, ch * N], f32)
            ss_sb = constp.tile([128, ch * N], f32)
            mu_sb = constp.tile([128, 256], f32)  # [mask | mask] for pairs
            id_sb = constp.tile([128, 128], bf)
            nc.sync.dma_start(
                cc_sb.rearrange("p (c n) -> p c n", c=ch),
                cc.rearrange("(c p) n -> p c n", p=128),
            )
            nc.sync.dma_start(
                ss_sb.rearrange("p (c n) -> p c n", c=ch),
                ss.rearrange("(c p) n -> p c n", p=128),
            )
            nc.sync.dma_start(mu_sb[:], mu[:])
            nc.sync.dma_start(id_sb[:], ident[:])

            for h in range(hpc):
                q_sb = headp.tile([128, ch * N], f32, tag="q")
                v_sb = headp.tile([128, ch * N], bf, tag="v")
                qr = headp.tile([128, ch * N], bf, tag="qr")
                qrt = headp.tile([64, ch * 128], bf, tag="qrt")

                q3 = q_sb.rearrange("p (c n) -> p c n", c=ch)
                v3 = v_sb.rearrange("p (c n) -> p c n", c=ch)

                # load + rope, pipelined per quarter-head
                for qt in range(nq):
                    csl = slice(qt * qtr, (qt + 1) * qtr)
                    fsl = slice(qt * qtr * N, (qt + 1) * qtr * N)
                    nc.sync.dma_start(q3[:, csl], q[h].rearrange(
                        "(c p) n -> p c n", p=128)[:, csl])
                    # SWDGE casts f32 -> bf16 during the transfer
                    nc.gpsimd.dma_start(v3[:, csl], v[h].rearrange(
                        "(c p) n -> p c n", p=128)[:, csl])

                    # swap(Q): exchange feature pairs, cast to bf16 (GpSimd)
                    swp = ropep.tile([128, qtr * N], bf, tag="swp")
                    sw4 = swp.rearrange("p (c m o) -> p c m o", c=qtr, m=32, o=2)
                    q4 = q3[:, csl].rearrange("p c (m o) -> p c m o", m=32)
                    nc.gpsimd.tensor_copy(sw4[:, :, :, 0], q4[:, :, :, 1])
                    nc.gpsimd.tensor_copy(sw4[:, :, :, 1], q4[:, :, :, 0])

                    # QR = Q*CC + swap(Q)*SS   (contiguous DVE ops)
                    t1 = ropep.tile([128, qtr * N], f32, tag="t1")
                    t2 = ropep.tile([128, qtr * N], f32, tag="t2")
                    nc.vector.tensor_mul(t1[:], q_sb[:, fsl], cc_sb[:, fsl])
                    nc.vector.tensor_mul(t2[:], swp[:], ss_sb[:, fsl])
                    nc.vector.tensor_add(qr[:, fsl], t1[:], t2[:])

                m_ps = psmp.tile([128, 64], f32, tag="m")  # fp32 state
                mb_prev = None
                ost = None
                for cp in range(ch // 2):
                    c0, c1 = 2 * cp, 2 * cp + 1

                    # QR^T strips for the pair via PE transpose, one ACT copy
                    tr_ps = psp.tile([64, 256], bf, tag="tr")
                    nc.tensor.transpose(
                        tr_ps[:, 0:128], qr[:, c0 * 64:(c0 + 1) * 64], id_sb[:]
                    )
                    nc.tensor.transpose(
                        tr_ps[:, 128:256], qr[:, c1 * 64:(c1 + 1) * 64], id_sb[:]
                    )
                    nc.scalar.copy(qrt[:, cp * 256:(cp + 1) * 256], tr_ps[:])

                    # intra: P blocks for both chunks into one PSUM tile,
                    # one masked copy (strict-upper mask doubled)
                    p_ps = psp.tile([128, 256], f32, tag="p")
                    p_sb = workp.tile([128, 256], bf, tag="psb")
                    out_ps = psp.tile([128, 128], f32, tag="out")
                    for k, c in ((0, c0), (1, c1)):
                        qrt_c = qrt[:, c * 128:(c + 1) * 128]
                        nc.tensor.matmul(
                            p_ps[:, k * 128:(k + 1) * 128], qrt_c, qrt_c,
                            start=True, stop=True,
                        )
                    nc.vector.tensor_mul(p_sb[:], p_ps[:], mu_sb[:])

                    for k, c in ((0, c0), (1, c1)):
                        qrt_c = qrt[:, c * 128:(c + 1) * 128]
                        v_c = v_sb[:, c * 64:(c + 1) * 64]
                        qr_c = qr[:, c * 64:(c + 1) * 64]
                        osl = slice(k * 64, (k + 1) * 64)
                        if c == 0:
                            nc.tensor.matmul(
                                out_ps[:, osl], p_sb[:, k * 128:(k + 1) * 128],
                                v_c, start=True, stop=True,
                            )
                        else:
                            # inter: out += QR_c @ M  (M = state after c-1)
                            nc.tensor.matmul(
                                out_ps[:, osl], qrt_c, mb_prev[0:64, :],
                                start=True, stop=False,
                            )
                            nc.tensor.matmul(
                                out_ps[:, osl], p_sb[:, k * 128:(k + 1) * 128],
                                v_c, start=False, stop=True,
                            )

                        # state: M += QR_c^T @ V_c, accumulated in PSUM
                        nc.tensor.matmul(
                            m_ps[0:64, :], qr_c, v_c,
                            start=(c == 0), stop=(c == ch - 1),
                            skip_group_check=True,
                        )
                        if c < ch - 1:
                            m_bf = workp.tile([64, 64], bf, tag="mbf")
                            nc.scalar.copy(m_bf[:], m_ps[0:64, :])
                            mb_prev = m_bf

                    # batch output: stage 4 chunks (2 pairs), then one DMA
                    k2 = cp % 2
                    if k2 == 0:
                        ost = workp.tile([128, 256], f32, tag="ost")
                    nc.scalar.copy(ost[:, k2 * 128:(k2 + 1) * 128], out_ps[:])
                    if k2 == 1:
                        g4 = cp // 2 * 4
                        nc.sync.dma_start(
                            o[h].rearrange("(g p) n -> p g n", p=128)[
                                :, g4:g4 + 4],
                            ost.rearrange("p (g n) -> p g n", g=4),
                        )

    nc.compile()
    return nc


_CACHE = {}


def _get_program():
    if "nc" not in _CACHE:
        _CACHE["nc"] = build_program()
    return _CACHE["nc"]


def _strict_upper_mask():
    # lhsT for the diag block: keep P[j, i] where j < i; doubled for pairs
    m = np.triu(np.ones((128, 128), dtype=np.float32), k=1)
    return np.ascontiguousarray(np.concatenate([m, m], axis=1))


def _identity():
    import ml_dtypes

    return np.eye(128, dtype=ml_dtypes.bfloat16)


def kernel(Q, V):
    from concourse.bass_utils import run_bass_kernel_spmd

    Q = np.ascontiguousarray(np.asarray(Q), dtype=np.float32)
    V = np.ascontiguousarray(np.asarray(V), dtype=np.float32)
    qf = Q.reshape(NCORES, HPC, T, N)
    vf = V.reshape(NCORES, HPC, T, N)
    cc, ss = _host_tables(T)
    mu = _strict_upper_mask()
    ident = _identity()

    nc = _get_program()
    in_maps = [
        {"q": qf[i], "v": vf[i], "cc": cc, "ss": ss, "mu": mu, "ident": ident}
        for i in range(NCORES)
    ]
    res = run_bass_kernel_spmd(nc, in_maps, core_ids=list(range(NCORES)))
    out = np.stack([r["o"] for r in res.results], axis=0)
    return out.reshape(B, H, T, N)


# revision 38
# speedup vs baseline: 1.4725x; 1.0630x over previous
"""Trainium2 Bass kernel for nn_Attention_23424751632639.

Computation (per (b,h)):  out = tril_strict(rope(Q) @ rope(Q).T / sqrt(N)) @ V
Reformulated as chunked linear attention (exact, just reordered sums):
  out_c = QR_c @ M_c  +  strict_mask(QR_c @ QR_c^T) @ V_c
  M_{c+1} = M_c + QR_c^T @ V_c            (M is the [64,64] running state)
with QR = rope(Q) * N**-0.25 (scale folded into the cos/sin tables, so the
score scale N**-0.5 appears automatically in both the intra and inter terms).

RoPE is computed as  QR = Q*CC + swap(Q)*SS  where swap exchanges feature
pairs (2m <-> 2m+1) and the rotation sign is folded into SS.  swap runs on
GpSimd (otherwise idle), the three wide elementwise ops on DVE.

Matmul operands are bf16 (PE 1 cyc/row vs 4 for fp32); all accumulation
(PSUM, the M state) stays fp32.  QR^T strips come from PE transposes.
Chunks are processed in pairs sharing PSUM tiles so each DVE/ACT fixup op
runs once per pair at double width.

Sharding: B*H = 32 (b,h) pairs -> 4 per core across 8 cores; no collectives.
"""

import math
import sys

import numpy as np

if "/opt/trn_rl_repo" not in sys.path:
    sys.path.insert(0, "/opt/trn_rl_repo")

B, H, T, N = 2, 16, 4096, 64
THETA = 2.0 ** 16
NCORES = 8
HPC = (B * H) // NCORES  # heads per core
QTR = 8  # chunks per pipeline stage (quarter-head granularity)


def _host_tables(t_len):
    """Full-width scaled RoPE tables CC, SS [t_len, N] float32.

    QR[t,n] = Q[t,n]*CC[t,n] + swap(Q)[t,n]*SS[t,n]
    where swap(Q)[2m] = Q[2m+1], swap(Q)[2m+1] = Q[2m]; the rotation minus
    sign lives in SS's even columns.
    """
    n = np.arange(N, dtype=np.float64)
    tq = np.floor(n / 2.0) * 2.0
    freqs = 1.0 / (THETA ** (tq / N)) / (2.0 * math.pi)  # [N]
    t = np.arange(t_len, dtype=np.float64)[:, None]
    ang = ((t * freqs[None, :]) % 1.0) * (2.0 * math.pi)  # [t_len, N]
    scale = float(N) ** -0.25
    cc = (np.cos(ang) * scale).astype(np.float32)
    ss = (np.sin(ang) * scale).astype(np.float32)
    ss[:, 0::2] *= -1.0
    return np.ascontiguousarray(cc), np.ascontiguousarray(ss)


def build_program(t_len=T, hpc=HPC):
    import concourse.mybir as mybir
    import concourse.tile as tile
    from concourse import bacc

    f32 = mybir.dt.float32
    bf = mybir.dt.bfloat16
    ch = t_len // 128  # number of 128-row chunks per head
    qtr = min(QTR, ch)  # chunks per pipeline stage
    nq = ch // qtr  # pipeline stages per head

    nc = bacc.Bacc(None, target_bir_lowering=False)
    q = nc.dram_tensor("q", [hpc, t_len, N], f32, kind="ExternalInput")
    v = nc.dram_tensor("v", [hpc, t_len, N], f32, kind="ExternalInput")
    cc = nc.dram_tensor("cc", [t_len, N], f32, kind="ExternalInput")
    ss = nc.dram_tensor("ss", [t_len, N], f32, kind="ExternalInput")
    mu = nc.dram_tensor("mu", [128, 256], f32, kind="ExternalInput")
    ident = nc.dram_tensor("ident", [128, 128], bf, kind="ExternalInput")
    o = nc.dram_tensor("o", [hpc, t_len, N], f32, kind="ExternalOutput")

    with tile.TileContext(nc) as tc:
        with (
            tc.tile_pool(name="const", bufs=1) as constp,
            tc.tile_pool(name="head", bufs=2) as headp,
            tc.tile_pool(name="rope", bufs=3) as ropep,
            tc.tile_pool(name="work", bufs=3) as workp,
            tc.tile_pool(name="ps", bufs=2, space="PSUM") as psp,
            tc.tile_pool(name="psm", bufs=2, space="PSUM") as psmp,
        ):
            cc_sb = constp.tile([128, ch * N], f32)
            ss_sb = constp.tile([128, ch * N], f32)
            mu_sb = constp.tile([128, 256], f32)  # [mask | mask] for pairs
            id_sb = constp.tile([128, 128], bf)
            nc.sync.dma_start(
                cc_sb.rearrange("p (c n) -> p c n", c=ch),
                cc.rearrange("(c p) n -> p c n", p=128),
            )
            nc.sync.dma_start(
                ss_sb.rearrange("p (c n) -> p c n", c=ch),
                ss.rearrange("(c p) n -> p c n", p=128),
            )
            nc.sync.dma_start(mu_sb[:], mu[:])
            nc.sync.dma_start(id_sb[:], ident[:])

            for h in range(hpc):
                q_sb = headp.tile([128, ch * N], f32, tag="q")
                v32 = headp.tile([128, ch * N], f32, tag="v32")
                v_sb = headp.tile([128, ch * N], bf, tag="v")
                qr = headp.tile([128, ch * N], bf, tag="qr")
                qrt = headp.tile([64, ch * 128], bf, tag="qrt")

                q3 = q_sb.rearrange("p (c n) -> p c n", c=ch)
                v3 = v32.rearrange("p (c n) -> p c n", c=ch)

                # load + rope, pipelined per quarter-head
                for qt in range(nq):
                    csl = slice(qt * qtr, (qt + 1) * qtr)
                    fsl = slice(qt * qtr * N, (qt + 1) * qtr * N)
                    nc.sync.dma_start(q3[:, csl], q[h].rearrange(
                        "(c p) n -> p c n", p=128)[:, csl])
                    # HWDGE f32 load (SWDGE cast-DMA is ~10x slower), then
                    # a DVE cast to bf16
                    nc.sync.dma_start(v3[:, csl], v[h].rearrange(
                        "(c p) n -> p c n", p=128)[:, csl])
                    nc.vector.tensor_copy(v_sb[:, fsl], v32[:, fsl])

                    # swap(Q): exchange feature pairs, cast to bf16 (GpSimd)
                    swp = ropep.tile([128, qtr * N], bf, tag="swp")
                    sw4 = swp.rearrange("p (c m o) -> p c m o", c=qtr, m=32, o=2)
                    q4 = q3[:, csl].rearrange("p c (m o) -> p c m o", m=32)
                    nc.gpsimd.tensor_copy(sw4[:, :, :, 0], q4[:, :, :, 1])
                    nc.gpsimd.tensor_copy(sw4[:, :, :, 1], q4[:, :, :, 0])

                    # QR = Q*CC + swap(Q)*SS   (contiguous DVE ops)
                    t1 = ropep.tile([128, qtr * N], f32, tag="t1")
                    t2 = ropep.tile([128, qtr * N], f32, tag="t2")
                    nc.vector.tensor_mul(t1[:], q_sb[:, fsl], cc_sb[:, fsl])
                    nc.vector.tensor_mul(t2[:], swp[:], ss_sb[:, fsl])
                    nc.vector.tensor_add(qr[:, fsl], t1[:], t2[:])

                m_ps = psmp.tile([128, 64], f32, tag="m")  # fp32 state
                mb_prev = None
                ost = None
                for cp in range(ch // 2):
                    c0, c1 = 2 * cp, 2 * cp + 1

                    # QR^T strips for the pair via PE transpose, one ACT copy
                    tr_ps = psp.tile([64, 256], bf, tag="tr")
                    nc.tensor.transpose(
                        tr_ps[:, 0:128], qr[:, c0 * 64:(c0 + 1) * 64], id_sb[:]
                    )
                    nc.tensor.transpose(
                        tr_ps[:, 128:256], qr[:, c1 * 64:(c1 + 1) * 64], id_sb[:]
                    )
                    nc.scalar.copy(qrt[:, cp * 256:(cp + 1) * 256], tr_ps[:])

                    # intra: P blocks for both chunks into one PSUM tile,
                    # one masked copy (strict-upper mask doubled)
                    p_ps = psp.tile([128, 256], f32, tag="p")
                    p_sb = workp.tile([128, 256], bf, tag="psb")
                    out_ps = psp.tile([128, 128], f32, tag="out")
                    for k, c in ((0, c0), (1, c1)):
                        qrt_c = qrt[:, c * 128:(c + 1) * 128]
                        nc.tensor.matmul(
                            p_ps[:, k * 128:(k + 1) * 128], qrt_c, qrt_c,
                            start=True, stop=True,
                        )
                    nc.vector.tensor_mul(p_sb[:], p_ps[:], mu_sb[:])

                    for k, c in ((0, c0), (1, c1)):
                        qrt_c = qrt[:, c * 128:(c + 1) * 128]
                        v_c = v_sb[:, c * 64:(c + 1) * 64]
                        qr_c = qr[:, c * 64:(c + 1) * 64]
                        osl = slice(k * 64, (k + 1) * 64)
                        if c == 0:
                            nc.tensor.matmul(
                                out_ps[:, osl], p_sb[:, k * 128:(k + 1) * 128],
                                v_c, start=True, stop=True,
                            )
                        else:
                            # inter: out += QR_c @ M  (M = state after c-1)
                            nc.tensor.matmul(
                                out_ps[:, osl], qrt_c, mb_prev[0:64, :],
                                start=True, stop=False,
                            )
                            nc.tensor.matmul(
                                out_ps[:, osl], p_sb[:, k * 128:(k + 1) * 128],
                                v_c, start=False, stop=True,
                            )

                        # state: M += QR_c^T @ V_c, accumulated in PSUM
                        nc.tensor.matmul(
                            m_ps[0:64, :], qr_c, v_c,
                            start=(c == 0), stop=(c == ch - 1),
                            skip_group_check=True,
                        )
                        if c < ch - 1:
                            m_bf = workp.tile([64, 64], bf, tag="mbf")
                            nc.scalar.copy(m_bf[:], m_ps[0:64, :])
                            mb_prev = m_bf

                    # batch output: stage 4 chunks (2 pairs), then one DMA
                    k2 = cp % 2
                    if k2 == 0:
                        ost = workp.tile([128, 256], f32, tag="ost")
                    nc.scalar.copy(ost[:, k2 * 128:(k2 + 1) * 128], out_ps[:])
                    if k2 == 1:
                        g4 = cp // 2 * 4
                        nc.sync.dma_start(
                            o[h].rearrange("(g p) n -> p g n", p=128)[
                                :, g4:g4 + 4],
                            ost.rearrange("p (g n) -> p g n", g=4),
                        )

    nc.compile()
    return nc


_CACHE = {}


def _get_program():
    if "nc" not in _CACHE:
        _CACHE["nc"] = build_program()
    return _CACHE["nc"]


def _strict_upper_mask():
    # lhsT for the diag block: keep P[j, i] where j < i; doubled for pairs
    m = np.triu(np.ones((128, 128), dtype=np.float32), k=1)
    return np.ascontiguousarray(np.concatenate([m, m], axis=1))


def _identity():
    import ml_dtypes

    return np.eye(128, dtype=ml_dtypes.bfloat16)


def kernel(Q, V):
    from concourse.bass_utils import run_bass_kernel_spmd

    Q = np.ascontiguousarray(np.asarray(Q), dtype=np.float32)
    V = np.ascontiguousarray(np.asarray(V), dtype=np.float32)
    qf = Q.reshape(NCORES, HPC, T, N)
    vf = V.reshape(NCORES, HPC, T, N)
    cc, ss = _host_tables(T)
    mu = _strict_upper_mask()
    ident = _identity()

    nc = _get_program()
    in_maps = [
        {"q": qf[i], "v": vf[i], "cc": cc, "ss": ss, "mu": mu, "ident": ident}
        for i in range(NCORES)
    ]
    res = run_bass_kernel_spmd(nc, in_maps, core_ids=list(range(NCORES)))
    out = np.stack([r["o"] for r in res.results], axis=0)
    return out.reshape(B, H, T, N)
